# revision 10
# baseline (speedup 1.0000x reference)
"""Trainium2 Bass kernel for nn_ContinuousNormalizingFlow.

Reference semantics (per step i, t_i = i/STEPS, dt = 1/STEPS):
    fwd at x_i:   h1 = tanh(x_i@W1x^T + t_i*w1t + b1); h2 = tanh(h1@W2^T + b2)
                  x_{i+1} = x_i + dt*(h2@W3^T + b3)
    vjp at x_{i+1} (same t_i): h1v, h2v as above at x_{i+1}
                  s = sum_k w1s[k] * (1-h1v_k^2) * sum_h W2[h,k]*w3sum[h]*(1-h2v_h^2)
                  ld += dt*s
Restructure used here (validated in numpy to ~1e-6 rel err):
  * feature-major on chip: [features(part), batch(free)], batch split 8-way across
    cores; each core's 4096 rows stacked as two 64-feature halves to fill 128
    partitions; G=4 independent column groups pipelined against each other.
  * state u = x - i*dt*b3 (b3 deferred, fixed on host); u kept in fp32, with an
    fp32r working copy (Pool engine) for the K=16 layer-1 matmul.
  * all layer-1 biases (b1 + t_i*w1t + i*(W1x@dt*b3)) are host-precomputed
    per-step ACT bias columns; layer-2 bias rides the tanh too.
  * a1v matmul of step i doubles as step i+1's layer-1 preactivation (only the
    ACT bias column differs).
  * log-det: distribute (1-sq1)(c-MG): ld accumulates entirely in PSUM via three
    M=2 matmuls per group (weights sw_sq2/sw_sq1/sw_r with dt folded); emitted
    one iteration late so they sit off the critical path. Host adds STEPS*dt*C1.
  * trace-only tensors (h1v,h2v,squares,rt + their matmuls) run in bf16.
"""

import numpy as np
import ml_dtypes

import concourse.bass as bass
import concourse.bacc as bacc
import concourse.tile as tile
from concourse import mybir
from concourse.bass_utils import run_bass_kernel_spmd

F32 = mybir.dt.float32
F32R = mybir.dt.float32r
BF16 = mybir.dt.bfloat16

# problem constants (hardcoded per contest contract)
B, D, H = 32768, 8, 64
NCORES = 8
BC = B // NCORES          # rows per core = 4096
SC = BC // 2              # stacked cols per core = 2048
W = 512                   # cols per group (one PSUM bank per matmul)
G = SC // W               # 4 column groups
P = 128

LAST_RESULTS = None       # BassKernelResults of the most recent run (for test.py)

_cache = {}


def _build_bass(steps: int):
    nc = bacc.Bacc()

    xin = nc.dram_tensor("xin", [2 * D, SC], F32, kind="ExternalInput")
    wf1 = nc.dram_tensor("wf1", [2 * D, P], F32R, kind="ExternalInput")
    wf2 = nc.dram_tensor("wf2", [P, P], F32R, kind="ExternalInput")
    wf3 = nc.dram_tensor("wf3", [P, 2 * D], F32R, kind="ExternalInput")
    wf2b = nc.dram_tensor("wf2b", [P, P], BF16, kind="ExternalInput")
    wgb = nc.dram_tensor("wgb", [P, P], BF16, kind="ExternalInput")
    wsb = nc.dram_tensor("wsb", [P, 96], BF16, kind="ExternalInput")
    bft = nc.dram_tensor("biasf", [P, steps], F32, kind="ExternalInput")
    bvt = nc.dram_tensor("biasv", [P, steps], F32, kind="ExternalInput")
    b2t = nc.dram_tensor("b2c", [P, 1], F32, kind="ExternalInput")
    xout = nc.dram_tensor("xout", [2 * D, SC], F32, kind="ExternalOutput")
    ldout = nc.dram_tensor("ldout", [P, 512], F32, kind="ExternalOutput")

    TANH = mybir.ActivationFunctionType.Tanh

    with tile.TileContext(nc) as tc:
        with (
            tc.tile_pool(name="consts", bufs=1) as consts,
            tc.tile_pool(name="state", bufs=1) as state,
            tc.tile_pool(name="acts", bufs=2) as acts,
            tc.tile_pool(name="chain", bufs=1, space="PSUM") as chain,
            tc.tile_pool(name="ldp", bufs=1, space="PSUM") as ldp,
        ):
            # ---- constants ----
            wf1_s = consts.tile([2 * D, P], F32R, tag="wf1")
            wf2_s = consts.tile([P, P], F32R, tag="wf2")
            wf3_s = consts.tile([P, 2 * D], F32R, tag="wf3")
            wf2b_s = consts.tile([P, P], BF16, tag="wf2b")
            wgb_s = consts.tile([P, P], BF16, tag="wgb")
            wsb_s = consts.tile([P, 96], BF16, tag="wsb")
            bf_s = consts.tile([P, steps], F32, tag="bf")
            bv_s = consts.tile([P, steps], F32, tag="bv")
            b2_s = consts.tile([P, 1], F32, tag="b2")
            for sb, dr in [
                (wf1_s, wf1), (wf2_s, wf2), (wf3_s, wf3), (wf2b_s, wf2b),
                (wgb_s, wgb), (wsb_s, wsb), (bf_s, bft), (bv_s, bvt), (b2_s, b2t),
            ]:
                nc.gpsimd.dma_start(out=sb[:], in_=dr[:])

            # ---- state: u in fp32; ur = fp32r working copy for matmuls ----
            u, ur = [], []
            for g in range(G):
                ug = state.tile([2 * D, W], F32, tag=f"u{g}", name=f"u{g}")
                nc.gpsimd.dma_start(out=ug[:], in_=xin[:, g * W:(g + 1) * W])
                u.append(ug)
                urg = state.tile([2 * D, W], F32R, tag=f"ur{g}", name=f"ur{g}")
                nc.gpsimd.tensor_copy(out=urg[:], in_=ug[:])
                ur.append(urg)

            ld_ps = [
                ldp.tile([32, W], F32, tag=f"ld{g}", name=f"ld{g}")
                for g in range(G)
            ]

            # ---- step 0 prologue: h1_0 = tanh(W1x@u0 + bias_f[0]) ----
            h1 = [None] * G
            for g in range(G):
                a1 = chain.tile([P, W], F32, tag=f"chain{g}")
                nc.tensor.matmul(a1[:], wf1_s[:], ur[g][:], start=True, stop=True)
                h1g = acts.tile([P, W], F32R, tag=f"h1g{g}")
                nc.scalar.activation(h1g[:], a1[:], TANH, bias=bf_s[:, 0:1])
                h1[g] = h1g

            pend = [None] * G  # deferred log-det matmul operands (prev step)

            def emit_ld_mms(g):
                sq2p, sq1p, rtp, i0 = pend[g]
                out_ap = ld_ps[g][:, :]
                nc.tensor.matmul(out_ap, wsb_s[:, 0:32], sq2p[:],
                                 start=(i0 == 0), stop=False)
                nc.tensor.matmul(out_ap, wsb_s[:, 32:64], sq1p[:],
                                 start=False, stop=False)
                nc.tensor.matmul(out_ap, wsb_s[:, 64:96], rtp[:],
                                 start=False, stop=(i0 == steps - 1))
                pend[g] = None

            # ---- main loop ----
            for i in range(steps):
                for g in range(G):
                    # fwd layer 2
                    a2 = chain.tile([P, W], F32, tag=f"chain{g}")
                    nc.tensor.matmul(a2[:], wf2_s[:], h1[g][:], start=True, stop=True)
                    if pend[g] is not None:
                        emit_ld_mms(g)
                    h2 = acts.tile([P, W], F32R, tag=f"h2g{g}")
                    nc.scalar.activation(h2[:], a2[:], TANH, bias=b2_s[:, 0:1])
                    # dx (dt, b3 folded) and state update
                    dx = chain.tile([P, W], F32, tag=f"chain{g}")
                    nc.tensor.matmul(dx[:2 * D, :], wf3_s[:], h2[:], start=True, stop=True)
                    nc.vector.tensor_add(u[g][:], u[g][:], dx[:2 * D, :])
                    nc.gpsimd.tensor_copy(out=ur[g][:], in_=u[g][:])
                    # vjp layer 1 (doubles as next step's fwd layer 1)
                    a1v = chain.tile([P, W], F32, tag=f"chain{g}")
                    nc.tensor.matmul(a1v[:], wf1_s[:], ur[g][:], start=True, stop=True)
                    h1v = acts.tile([P, W], BF16, tag=f"h1v{g}")
                    nc.scalar.activation(h1v[:], a1v[:], TANH, bias=bv_s[:, i:i + 1])
                    if i + 1 < steps:
                        h1n = acts.tile([P, W], F32R, tag=f"h1g{g}")
                        nc.scalar.activation(h1n[:], a1v[:], TANH, bias=bf_s[:, i + 1:i + 2])
                        h1[g] = h1n
                    # vjp layer 2
                    a2v = chain.tile([P, W], F32, tag=f"chain{g}")
                    nc.tensor.matmul(a2v[:], wf2b_s[:], h1v[:], start=True, stop=True)
                    h2v = acts.tile([P, W], BF16, tag=f"h2v{g}")
                    nc.scalar.activation(h2v[:], a2v[:], TANH, bias=b2_s[:, 0:1])
                    # squares (bf16, DVE 2x) and trace pieces
                    sq2 = acts.tile([P, W], BF16, tag=f"sq2{g}")
                    nc.vector.tensor_mul(sq2[:], h2v[:], h2v[:])
                    mg = chain.tile([P, W], F32, tag=f"chain{g}")
                    nc.tensor.matmul(mg[:], wgb_s[:], sq2[:], start=True, stop=True)
                    sq1 = acts.tile([P, W], BF16, tag=f"sq1{g}")
                    nc.vector.tensor_mul(sq1[:], h1v[:], h1v[:])
                    rt = acts.tile([P, W], BF16, tag=f"rt{g}")
                    nc.vector.tensor_mul(rt[:], sq1[:], mg[:])
                    pend[g] = (sq2, sq1, rt, i)

            for g in range(G):
                emit_ld_mms(g)

            # ---- epilogue ----
            ldsb = acts.tile([P, 512], F32, tag="ldsb")
            for g in range(G):
                nc.scalar.copy(ldsb[32 * g:32 * g + 32, :], ld_ps[g][:, :])
            nc.gpsimd.dma_start(out=ldout[:], in_=ldsb[:])
            for g in range(G):
                nc.gpsimd.dma_start(out=xout[:, g * W:(g + 1) * W], in_=u[g][:])

    nc.compile()
    return nc


def _prep(W1, b1, W2, b2, W3, b3, steps):
    """Host-side precompute of folded weights/biases."""
    f = np.float32
    bf = ml_dtypes.bfloat16
    dt = f(1.0 / steps)
    W1x = W1[:, :D].astype(f)
    w1t = W1[:, D].astype(f)
    c3 = dt * b3.astype(f)
    wc3 = W1x @ c3
    idx = np.arange(steps, dtype=f)
    ts = idx * dt
    bias_f = b1[None, :] + ts[:, None] * w1t[None, :] + idx[:, None] * wc3[None, :]
    bias_v = bias_f + wc3[None, :]
    w3sum = W3.sum(axis=0).astype(f)
    Wg = (W2 * w3sum[:, None]).astype(f)
    c = Wg.sum(axis=0)
    w1s = W1x.sum(axis=1)
    w2t = Wg @ w1s
    C1 = f(w1s @ c)

    wf1 = np.zeros((2 * D, P), f)
    wf1[0:D, 0:H] = W1x.T
    wf1[D:2 * D, H:2 * H] = W1x.T
    wf2 = np.zeros((P, P), f)
    wf2[0:H, 0:H] = W2.T
    wf2[H:2 * H, H:2 * H] = W2.T
    wf3 = np.zeros((P, 2 * D), f)
    wf3[0:H, 0:D] = (dt * W3).T
    wf3[H:2 * H, D:2 * D] = (dt * W3).T
    wg = np.zeros((P, P), f)
    wg[0:H, 0:H] = Wg
    wg[H:2 * H, H:2 * H] = Wg
    ws = np.zeros((P, 96), f)
    ws[0:H, 0] = -dt * w2t
    ws[H:2 * H, 1] = -dt * w2t
    ws[0:H, 32] = -dt * (w1s * c)
    ws[H:2 * H, 33] = -dt * (w1s * c)
    ws[0:H, 64] = dt * w1s
    ws[H:2 * H, 65] = dt * w1s
    bft = np.zeros((P, steps), f)
    bft[0:H] = bias_f.T
    bft[H:2 * H] = bias_f.T
    bvt = np.zeros((P, steps), f)
    bvt[0:H] = bias_v.T
    bvt[H:2 * H] = bias_v.T
    b2c = np.zeros((P, 1), f)
    b2c[0:H, 0] = b2
    b2c[H:2 * H, 0] = b2

    consts = dict(
        wf1=wf1, wf2=wf2, wf3=wf3,
        wf2b=wf2.astype(bf), wgb=wg.astype(bf), wsb=ws.astype(bf),
        biasf=bft, biasv=bvt, b2c=b2c,
    )
    return consts, c3, C1, dt


def kernel(x0, W1, b1, W2, b2, W3, b3, num_steps, trace=False):
    global LAST_RESULTS
    steps = int(num_steps)
    x0 = np.ascontiguousarray(np.asarray(x0, np.float32))
    assert x0.shape == (B, D)

    consts, c3, C1, dt = _prep(
        np.asarray(W1, np.float32), np.asarray(b1, np.float32),
        np.asarray(W2, np.float32), np.asarray(b2, np.float32),
        np.asarray(W3, np.float32), np.asarray(b3, np.float32), steps)

    if steps not in _cache:
        _cache[steps] = _build_bass(steps)
    nc = _cache[steps]

    in_maps = []
    for ci in range(NCORES):
        sh = x0[ci * BC:(ci + 1) * BC]               # [4096, 8]
        xin = np.empty((2 * D, SC), np.float32)
        xin[0:D] = sh[0:SC].T
        xin[D:2 * D] = sh[SC:2 * SC].T
        m = dict(consts)
        m["xin"] = np.ascontiguousarray(xin)
        in_maps.append(m)

    res = run_bass_kernel_spmd(nc, in_maps, core_ids=list(range(NCORES)), trace=trace)
    LAST_RESULTS = res

    x = np.empty((B, D), np.float32)
    ld = np.empty(B, np.float32)
    shift = np.float32(steps) * c3
    ldc = np.float32(steps) * dt * C1
    for ci in range(NCORES):
        out = res.results[ci]
        xo = out["xout"]
        x[ci * BC:ci * BC + SC] = xo[0:D].T + shift
        x[ci * BC + SC:(ci + 1) * BC] = xo[D:2 * D].T + shift
        lo = out["ldout"]
        for g in range(G):
            base = ci * BC + g * W
            ld[base:base + W] = lo[32 * g + 0, :] + ldc
            ld[base + SC:base + SC + W] = lo[32 * g + 1, :] + ldc
    return x, ld
